# revision 34
# baseline (speedup 1.0000x reference)
"""Trainium2 Bass kernel for BlockSelector (sparse-attention block-index masks).

Math (from the reference):
    i            = arange(S)
    cur_block    = i // block_size
    self_start   = cur_block * block_size             broadcast to [B,H,S]
    self_end     = min(i+1, (cur_block+1)*block_size) == i+1
    moba_valid   = topk < cur_block                   [B,H,S,K]
    moba_start   = where(valid, topk*block_size, 0)
    moba_end     = where(valid, (topk+1)*block_size, 0)

Only `topk_indices` is ever read; q/k/v are untouched.  self_start/self_end
are compile-time constants -> embedded in the NEFF as one [2,PPC,S] tensor
and copied DRAM->DRAM in a single DMA (split host-side).

Sharding: embarrassingly parallel over B*H = 32 (b,h) pairs -> 4 pairs per
core on 8 cores.  Layout: partition p = j*32 + c  (j = pair in [0,4),
c = i//128 in [0,32)), free dim = the 128*K contiguous elements of
(i%128, k).  That makes every big DMA a uniform 2-D access pattern
(partition stride = 1536B contiguous payload), the cheapest shape for
HWDGE descriptor generation.  cur_block = (p%32)//4 is not affine in p, so
DVE derives it ([128,1]: Pool iota p, then &31 -> >>2 -> f32) before the
input arrives; the validity mask is then one tensor_scalar per-partition
f32 compare (the HW requires AP scalars in f32; exact for these small
ints).

Engine plan (raw bass, explicit semaphores, no Tile / no exit barrier —
the end-of-block all-engine barrier and Tile's tail drain are pure
overhead for a one-shot DAG and the drain exceeds the CoreV3 sync-wait
slot limit here anyway):
    SP  : input half A -> moba_end store (the last-ready tensor gets the
          otherwise-empty ring so its transfer queues behind nothing)
    ACT : input half B -> self-const DRAM->DRAM (after the input lands so
          the load has HBM to itself) -> moba_valid -> moba_start stores,
          in data-ready order so nothing queues behind a later tensor
    DVE : cb prep (hidden under input DMA); valid(u8) -> moba_start ->
          moba_end   (valid is computed straight into the u8 output form;
          the int32 arithmetic reads the u8 mask directly)
    Pool: iota only (Pool cannot run TensorScalar ops, and its SBUF-port
          contention slows DVE, so everything else stays off it)

No engine waits for the output DMAs to complete: the NEFF teardown emits
per-engine DRAINs that quiesce the DGE rings (NRT must drain them before
reusing), so the final transfers overlap the ~1 us teardown instead of
preceding it (worth ~1.6 us, outputs verified bit-exact across runs).
Measured ~13.9-14.2 us NEFF exec across all 8 cores on trn2, vs ~11.3 us
for an empty kernel (~7.5 us fixed preamble + ~1 us teardown); the
marginal ~2.7 us is per-DMA fixed latencies (descriptor-gen ~0.65 us,
trigger ~0.7 us, completion-sem ~0.5 us each) plus three serial DVE ops
and the 765KB per core of unavoidable HBM traffic.
"""

import numpy as np

import concourse.bass as bass
from concourse import mybir
from concourse.bass_utils import run_bass_kernel_spmd

B, H, S, K = 2, 16, 4096, 3
BLOCK = 512
NCORES = 8
PAIRS = B * H               # 32 (b,h) pairs
PPC = PAIRS // NCORES       # 4 pairs per core
P = 128                     # SBUF partitions
CH = 32                     # i-chunks of 128 per pair (S / 128)
M = S // CH                 # 128 consecutive seq positions per partition
F = M * K                   # 384 int32 elements per partition per pair
HA = 192                    # SP-ring input chunk; ACT ring loads the rest

_cached = {}


class _NoBarrierBlock(bass.BassBlock):
    """BassBlock without the exit-time all-engine barrier.

    The kernel is a one-shot static DAG with no semaphore reuse; output
    durability is guaranteed by the NEFF teardown's per-engine DGE
    drains, so the end-of-block barrier is pure overhead.
    """

    def __exit__(self, exc_type, exc_val, exc_tb):
        if exc_type is not None:
            return
        for engine, last_body in self.last_body.items():
            with self.bass.body(
                last_body, parent=self.bass.cur_bb, allow_existing_parent=True
            ):
                engine.br(self.end_bb)
        self.bass.switch_bb(self.end_bb)


def _self_consts():
    i = np.arange(S, dtype=np.int32)
    ss = np.broadcast_to((i // BLOCK) * BLOCK, (PPC, S))
    se = np.broadcast_to(i + 1, (PPC, S))
    return np.stack([ss, se]).copy()   # [2, PPC, S]


def _build():
    op = mybir.AluOpType
    i32 = mybir.dt.int32
    f32 = mybir.dt.float32
    u8 = mybir.dt.uint8

    nc = bass.Bass()
    tk_d = nc.dram_tensor("topk", [PPC, S, K], i32, kind="ExternalInput")
    sse_d = nc.dram_tensor("self_startend", [2, PPC, S], i32,
                           kind="ExternalOutput")
    ms_d = nc.dram_tensor("moba_start", [PPC, S, K], i32, kind="ExternalOutput")
    me_d = nc.dram_tensor("moba_end", [PPC, S, K], i32, kind="ExternalOutput")
    mv_d = nc.dram_tensor("moba_valid", [PPC, S, K], u8, kind="ExternalOutput")

    sse_c = nc.inline_tensor(_self_consts(), name="self_startend_const")

    # partition p = j*32 + c -> uniform 2D view [(j c), (m k)]
    def part_view(dram):
        return dram[:].rearrange("j (c m) k -> (j c) (m k)", c=CH)

    with (
        nc.sbuf_tensor("tk_sb", [P, F], i32) as tk_sb,
        nc.sbuf_tensor("pcol", [P, 1], i32) as pcol,
        nc.sbuf_tensor("pmod", [P, 1], i32) as pmod,
        nc.sbuf_tensor("pblk", [P, 1], i32) as pblk,
        nc.sbuf_tensor("cb_sb", [P, 1], f32) as cb_sb,
        nc.sbuf_tensor("mstart", [P, F], i32) as mstart,
        nc.sbuf_tensor("mend", [P, F], i32) as mend,
        nc.sbuf_tensor("validu8", [P, F], u8) as validu8,
        nc.semaphore("s_in") as s_in,
        nc.semaphore("s_gp") as s_gp,
        nc.semaphore("s_v") as s_v,
        nc.semaphore("s_out") as s_out,
        _NoBarrierBlock(nc, f"block_{nc.next_id()}") as block,
    ):
        nc.cur_block = block

        @block.gpsimd
        def _(gpsimd):
            gpsimd.iota(pcol[:], [[0, 1]], base=0,
                        channel_multiplier=1).then_inc(s_gp, 1)

        @block.vector
        def _(vector):
            # cb = (p % 32) // 4 as f32 per-partition scalar; no input
            # dependency, so this hides under the input DMA.
            vector.wait_ge(s_gp, 1)
            vector.tensor_scalar(
                pmod[:], pcol[:], 31, None, op0=op.bitwise_and
            ).then_inc(s_v, 1)
            vector.wait_ge(s_v, 1)
            vector.tensor_scalar(
                pblk[:], pmod[:], 2, None, op0=op.arith_shift_right
            ).then_inc(s_v, 1)
            vector.wait_ge(s_v, 2)
            vector.tensor_copy(cb_sb[:], pblk[:]).then_inc(s_v, 1)
            vector.wait_ge(s_in, 32)
            vector.wait_ge(s_v, 3)
            # valid = tk < cur_block(p), directly in u8 (doubles as output)
            vector.tensor_scalar(
                validu8[:], tk_sb[:], cb_sb[:], None, op0=op.is_lt
            ).then_inc(s_v, 1)
            vector.wait_ge(s_v, 4)
            # moba_start = (tk*512) * valid
            vector.scalar_tensor_tensor(
                mstart[:], tk_sb[:], BLOCK, validu8[:], op0=op.mult, op1=op.mult
            ).then_inc(s_v, 1)
            vector.wait_ge(s_v, 5)
            # moba_end = (valid*512) + moba_start
            vector.scalar_tensor_tensor(
                mend[:], validu8[:], BLOCK, mstart[:], op0=op.mult, op1=op.add
            ).then_inc(s_v, 1)

        @block.sync
        def _(sync):
            pv = part_view(tk_d)
            sync.dma_start(tk_sb[:, :HA], pv[:, :HA]).then_inc(s_in, 16)
            # moba_end is the last tensor ready -> it gets the otherwise-empty
            # SP ring so its transfer doesn't queue behind anything
            sync.wait_ge(s_v, 6)
            sync.dma_start(part_view(me_d), mend[:]).then_inc(s_out, 16)

        @block.scalar
        def _(scalar):
            scalar.dma_start(
                tk_sb[:, HA:], part_view(tk_d)[:, HA:]
            ).then_inc(s_in, 16)
            # after the input lands, so the load had HBM to itself
            scalar.wait_ge(s_in, 32)
            scalar.dma_start(sse_d[:], sse_c[:]).then_inc(s_out, 16)
            scalar.wait_ge(s_v, 4)
            scalar.dma_start(part_view(mv_d), validu8[:]).then_inc(s_out, 16)
            scalar.wait_ge(s_v, 5)
            scalar.dma_start(part_view(ms_d), mstart[:]).then_inc(s_out, 16)

    nc.cur_block = None
    return nc


def _get_nc():
    if "nc" not in _cached:
        _cached["nc"] = _build()
    return _cached["nc"]


def kernel(q=None, k=None, v=None, topk_indices=None, query_block_indices=None,
           block_size=512, seq_len=4096, _run_kwargs=None, **_unused):
    tk = np.ascontiguousarray(np.asarray(topk_indices, dtype=np.int32))
    tk = tk.reshape(PAIRS, S, K)
    in_maps = [{"topk": tk[c * PPC:(c + 1) * PPC]} for c in range(NCORES)]

    nc = _get_nc()
    out = run_bass_kernel_spmd(nc, in_maps, list(range(NCORES)),
                               **(_run_kwargs or {}))
    res = out.results
    _cached["last_result"] = out

    def gather(name, shape, dtype, axis=0):
        full = np.concatenate(
            [np.asarray(res[c][name]) for c in range(NCORES)], axis=axis)
        return np.ascontiguousarray(full.reshape(shape).astype(dtype, copy=False))

    sse = gather("self_startend", (2, B, H, S), np.int32, axis=1)
    self_start = np.ascontiguousarray(sse[0])
    self_end = np.ascontiguousarray(sse[1])
    moba_start = gather("moba_start", (B, H, S, K), np.int32)
    moba_end = gather("moba_end", (B, H, S, K), np.int32)
    moba_valid = gather("moba_valid", (B, H, S, K), np.uint8).astype(bool)
    return (self_start, self_end, moba_start, moba_end, moba_valid)


# revision 35
# speedup vs baseline: 1.0364x; 1.0364x over previous
"""Trainium2 Bass kernel for BlockSelector (sparse-attention block-index masks).

Math (from the reference):
    i            = arange(S)
    cur_block    = i // block_size
    self_start   = cur_block * block_size             broadcast to [B,H,S]
    self_end     = min(i+1, (cur_block+1)*block_size) == i+1
    moba_valid   = topk < cur_block                   [B,H,S,K]
    moba_start   = where(valid, topk*block_size, 0)
    moba_end     = where(valid, (topk+1)*block_size, 0)

Only `topk_indices` is ever read; q/k/v are untouched.  self_start/self_end
are compile-time constants -> embedded in the NEFF as one [2,PPC,S] tensor
and copied DRAM->DRAM in a single DMA (split host-side).

Sharding: embarrassingly parallel over B*H = 32 (b,h) pairs -> 4 pairs per
core on 8 cores.  Layout: partition p = j*32 + c  (j = pair in [0,4),
c = i//128 in [0,32)), free dim = the 128*K contiguous elements of
(i%128, k).  That makes every big DMA a uniform 2-D access pattern
(partition stride = 1536B contiguous payload), the cheapest shape for
HWDGE descriptor generation.  cur_block = (p%32)//4 is not affine in p, so
DVE derives it ([128,1]: Pool iota p, then &31 -> >>2 -> f32) before the
input arrives; the validity mask is then one tensor_scalar per-partition
f32 compare (the HW requires AP scalars in f32; exact for these small
ints).

Engine plan (raw bass, explicit semaphores, no Tile / no exit barrier —
the end-of-block all-engine barrier and Tile's tail drain are pure
overhead for a one-shot DAG and the drain exceeds the CoreV3 sync-wait
slot limit here anyway):
    SP  : input half A -> moba_end store (the last-ready tensor gets the
          otherwise-empty ring so its transfer queues behind nothing)
    ACT : input half B -> self-const DRAM->DRAM (after the input lands so
          the load has HBM to itself) -> moba_valid -> moba_start stores,
          in data-ready order so nothing queues behind a later tensor
    DVE : cb prep (hidden under input DMA); valid(u8) -> moba_start ->
          moba_end   (valid is computed straight into the u8 output form;
          the int32 arithmetic reads the u8 mask directly)
    Pool: iota only (Pool cannot run TensorScalar ops, and its SBUF-port
          contention slows DVE, so everything else stays off it)

No engine waits for the output DMAs to complete: the NEFF teardown emits
per-engine DRAINs that quiesce the DGE rings (NRT must drain them before
reusing), so the final transfers overlap the ~1 us teardown instead of
preceding it (worth ~1.6 us, outputs verified bit-exact across runs).
Measured ~13.9-14.2 us NEFF exec across all 8 cores on trn2, vs ~11.3 us
for an empty kernel (~7.5 us fixed preamble + ~1 us teardown); the
marginal ~2.7 us is per-DMA fixed latencies (descriptor-gen ~0.65 us,
trigger ~0.7 us, completion-sem ~0.5 us each) plus three serial DVE ops
and the 765KB per core of unavoidable HBM traffic.
"""

import numpy as np

import concourse.bass as bass
from concourse import mybir
from concourse.bass_utils import run_bass_kernel_spmd

B, H, S, K = 2, 16, 4096, 3
BLOCK = 512
NCORES = 8
PAIRS = B * H               # 32 (b,h) pairs
PPC = PAIRS // NCORES       # 4 pairs per core
P = 128                     # SBUF partitions
CH = 32                     # i-chunks of 128 per pair (S / 128)
M = S // CH                 # 128 consecutive seq positions per partition
F = M * K                   # 384 int32 elements per partition per pair
HA = 256                    # SP-ring input chunk; ACT's starts later, so smaller

_cached = {}


class _NoBarrierBlock(bass.BassBlock):
    """BassBlock without the exit-time all-engine barrier.

    The kernel is a one-shot static DAG with no semaphore reuse; output
    durability is guaranteed by the NEFF teardown's per-engine DGE
    drains, so the end-of-block barrier is pure overhead.
    """

    def __exit__(self, exc_type, exc_val, exc_tb):
        if exc_type is not None:
            return
        for engine, last_body in self.last_body.items():
            with self.bass.body(
                last_body, parent=self.bass.cur_bb, allow_existing_parent=True
            ):
                engine.br(self.end_bb)
        self.bass.switch_bb(self.end_bb)


def _self_consts():
    i = np.arange(S, dtype=np.int32)
    ss = np.broadcast_to((i // BLOCK) * BLOCK, (PPC, S))
    se = np.broadcast_to(i + 1, (PPC, S))
    return np.stack([ss, se]).copy()   # [2, PPC, S]


def _build():
    op = mybir.AluOpType
    i32 = mybir.dt.int32
    f32 = mybir.dt.float32
    u8 = mybir.dt.uint8

    nc = bass.Bass()
    tk_d = nc.dram_tensor("topk", [PPC, S, K], i32, kind="ExternalInput")
    sse_d = nc.dram_tensor("self_startend", [2, PPC, S], i32,
                           kind="ExternalOutput")
    ms_d = nc.dram_tensor("moba_start", [PPC, S, K], i32, kind="ExternalOutput")
    me_d = nc.dram_tensor("moba_end", [PPC, S, K], i32, kind="ExternalOutput")
    mv_d = nc.dram_tensor("moba_valid", [PPC, S, K], u8, kind="ExternalOutput")

    sse_c = nc.inline_tensor(_self_consts(), name="self_startend_const")

    # partition p = j*32 + c -> uniform 2D view [(j c), (m k)]
    def part_view(dram):
        return dram[:].rearrange("j (c m) k -> (j c) (m k)", c=CH)

    with (
        nc.sbuf_tensor("tk_sb", [P, F], i32) as tk_sb,
        nc.sbuf_tensor("pcol", [P, 1], i32) as pcol,
        nc.sbuf_tensor("pmod", [P, 1], i32) as pmod,
        nc.sbuf_tensor("pblk", [P, 1], i32) as pblk,
        nc.sbuf_tensor("cb_sb", [P, 1], f32) as cb_sb,
        nc.sbuf_tensor("mstart", [P, F], i32) as mstart,
        nc.sbuf_tensor("mend", [P, F], i32) as mend,
        nc.sbuf_tensor("validu8", [P, F], u8) as validu8,
        nc.semaphore("s_in") as s_in,
        nc.semaphore("s_gp") as s_gp,
        nc.semaphore("s_v") as s_v,
        nc.semaphore("s_out") as s_out,
        _NoBarrierBlock(nc, f"block_{nc.next_id()}") as block,
    ):
        nc.cur_block = block

        @block.gpsimd
        def _(gpsimd):
            gpsimd.iota(pcol[:], [[0, 1]], base=0,
                        channel_multiplier=1).then_inc(s_gp, 1)

        @block.vector
        def _(vector):
            # cb = (p % 32) // 4 as f32 per-partition scalar; no input
            # dependency, so this hides under the input DMA.
            vector.wait_ge(s_gp, 1)
            vector.tensor_scalar(
                pmod[:], pcol[:], 31, None, op0=op.bitwise_and
            ).then_inc(s_v, 1)
            vector.wait_ge(s_v, 1)
            vector.tensor_scalar(
                pblk[:], pmod[:], 2, None, op0=op.arith_shift_right
            ).then_inc(s_v, 1)
            vector.wait_ge(s_v, 2)
            vector.tensor_copy(cb_sb[:], pblk[:]).then_inc(s_v, 1)
            vector.wait_ge(s_in, 32)
            vector.wait_ge(s_v, 3)
            # valid = tk < cur_block(p), directly in u8 (doubles as output)
            vector.tensor_scalar(
                validu8[:], tk_sb[:], cb_sb[:], None, op0=op.is_lt
            ).then_inc(s_v, 1)
            vector.wait_ge(s_v, 4)
            # moba_start = (tk*512) * valid
            vector.scalar_tensor_tensor(
                mstart[:], tk_sb[:], BLOCK, validu8[:], op0=op.mult, op1=op.mult
            ).then_inc(s_v, 1)
            vector.wait_ge(s_v, 5)
            # moba_end = (valid*512) + moba_start
            vector.scalar_tensor_tensor(
                mend[:], validu8[:], BLOCK, mstart[:], op0=op.mult, op1=op.add
            ).then_inc(s_v, 1)

        @block.sync
        def _(sync):
            pv = part_view(tk_d)
            sync.dma_start(tk_sb[:, :HA], pv[:, :HA]).then_inc(s_in, 16)
            # moba_end is the last tensor ready -> it gets the otherwise-empty
            # SP ring so its transfer doesn't queue behind anything
            sync.wait_ge(s_v, 6)
            sync.dma_start(part_view(me_d), mend[:]).then_inc(s_out, 16)

        @block.scalar
        def _(scalar):
            scalar.dma_start(
                tk_sb[:, HA:], part_view(tk_d)[:, HA:]
            ).then_inc(s_in, 16)
            # after the input lands, so the load had HBM to itself
            scalar.wait_ge(s_in, 32)
            scalar.dma_start(sse_d[:], sse_c[:]).then_inc(s_out, 16)
            scalar.wait_ge(s_v, 4)
            scalar.dma_start(part_view(mv_d), validu8[:]).then_inc(s_out, 16)
            scalar.wait_ge(s_v, 5)
            scalar.dma_start(part_view(ms_d), mstart[:]).then_inc(s_out, 16)

    nc.cur_block = None
    return nc


def _get_nc():
    if "nc" not in _cached:
        _cached["nc"] = _build()
    return _cached["nc"]


def kernel(q=None, k=None, v=None, topk_indices=None, query_block_indices=None,
           block_size=512, seq_len=4096, _run_kwargs=None, **_unused):
    tk = np.ascontiguousarray(np.asarray(topk_indices, dtype=np.int32))
    tk = tk.reshape(PAIRS, S, K)
    in_maps = [{"topk": tk[c * PPC:(c + 1) * PPC]} for c in range(NCORES)]

    nc = _get_nc()
    out = run_bass_kernel_spmd(nc, in_maps, list(range(NCORES)),
                               **(_run_kwargs or {}))
    res = out.results
    _cached["last_result"] = out

    def gather(name, shape, dtype, axis=0):
        full = np.concatenate(
            [np.asarray(res[c][name]) for c in range(NCORES)], axis=axis)
        return np.ascontiguousarray(full.reshape(shape).astype(dtype, copy=False))

    sse = gather("self_startend", (2, B, H, S), np.int32, axis=1)
    self_start = np.ascontiguousarray(sse[0])
    self_end = np.ascontiguousarray(sse[1])
    moba_start = gather("moba_start", (B, H, S, K), np.int32)
    moba_end = gather("moba_end", (B, H, S, K), np.int32)
    moba_valid = gather("moba_valid", (B, H, S, K), np.uint8).astype(bool)
    return (self_start, self_end, moba_start, moba_end, moba_valid)
